# revision 53
# baseline (speedup 1.0000x reference)
"""DynamicBottleneck Trainium2 kernel.

Data-parallel over batch: each of 8 NeuronCores computes one sample of
x: [8, 256, 80, 80] through conv3x3 -> GN -> ReLU -> conv3x3 -> GN,
a 1x1-conv spatial gate (ReTanH) on the input, gating + residual + ReLU.

Per-core layout: channels on partitions (256 = 2 chunks of 128), spatial
pixels on the free dim. Inputs/hidden live in SBUF as zero-padded 82x82
images so each of the 9 conv taps is a strided AP read; the conv is 9
accumulated f32r matmuls per (out-chunk, in-chunk) into PSUM. GroupNorm
stats come from bn_stats/bn_aggr per channel + a block-mask matmul for the
cross-partition (8-channel group) reduction.
"""

import sys

sys.path.insert(0, "/opt/trn_rl_repo")

import numpy as np
import concourse.bass as bass
import concourse.tile as tile
from concourse import mybir
from concourse.bass_utils import run_bass_kernel_spmd

f32 = mybir.dt.float32
f32r = mybir.dt.float32r
AF = mybir.ActivationFunctionType
ALU = mybir.AluOpType

B, C, H, W = 8, 256, 80, 80
HW = H * W          # 6400
PW = W + 2          # 82
PHW = PW * PW       # 6724
MC = C // 128       # output-channel chunks
KC = C // 128       # input-channel chunks
GROUP = 8           # channels per GN group (256 / 32)
R = 5               # image rows per spatial tile -> N=400 per matmul
TILES = [(y0, R) for y0 in range(0, H, R)]   # 16 equal tiles
NTILES = len(TILES)
CONV_GROUP = 5      # PSUM banks cycled by conv accumulation
XBANDS = 2          # row-bands for the staged x load
EPS = 1e-5

# ---------------------------------------------------------------------------
# walrus in this container accepts only ONE sem-wait per instruction; tile
# attaches several. Rewrite blocks so extra waits ride on single-wait NOPs.
_ENGINE_ATTR = {
    "EngineType.PE": "tensor",
    "EngineType.Activation": "scalar",
    "EngineType.DVE": "vector",
    "EngineType.Pool": "gpsimd",
    "EngineType.SP": "sync",
}


def _fresh_nop(nc, engine):
    bi = getattr(nc, _ENGINE_ATTR[str(engine)]).nop(nofuse=True)
    cur = nc.cur_bb.bb
    insts = cur.instructions
    assert insts and insts[-1].name == bi.ins.name
    cur.instructions = insts[:-1]
    return bi.ins


def _split_multi_waits(nc):
    for f in nc.m.functions:
        for bb in f.blocks:
            insts = bb.instructions
            if not any(
                i.sync_info is not None and len(i.sync_info.on_wait) > 1
                for i in insts
            ):
                continue
            out = []
            for inst in insts:
                si = inst.sync_info
                if si is not None and len(si.on_wait) > 1:
                    waits = list(si.on_wait)
                    for w in waits[:-1]:
                        nop = _fresh_nop(nc, inst.engine)
                        nop.sync_info = mybir.SyncInfo(on_wait=[w], on_update=[])
                        out.append(nop)
                    inst.sync_info = mybir.SyncInfo(
                        on_wait=[waits[-1]], on_update=list(si.on_update)
                    )
                out.append(inst)
            bb.instructions = out


# ---------------------------------------------------------------------------


def _pad_view(buf_ap, kc):
    """[128, PHW] padded-image chunk as [128, 82, 82]."""
    return buf_ap[:, kc, :].rearrange("p (r c) -> p r c", c=PW)


def build_program(gate_bias: float):
    nc = bass.Bass()

    x_h = nc.declare_dram_parameter("x", [C, HW], f32, isOutput=False)
    w1_h = nc.declare_dram_parameter("w1t", [9, C, C], f32, isOutput=False)
    w2_h = nc.declare_dram_parameter("w2t", [9, C, C], f32, isOutput=False)
    gn_h = {}
    for name in ("gn1w", "gn1b", "gn2w", "gn2b"):
        gn_h[name] = nc.declare_dram_parameter(name, [C], f32, isOutput=False)
    gatew_h = nc.declare_dram_parameter("gatew", [C], f32, isOutput=False)
    mask_h = nc.declare_dram_parameter("mask", [128, 128], f32, isOutput=False)
    ones_h = nc.declare_dram_parameter("ones", [1, 128], f32, isOutput=False)
    ident_h = nc.declare_dram_parameter("ident", [128, 128], f32, isOutput=False)
    y_h = nc.declare_dram_parameter("y", [C, HW], f32, isOutput=True)

    with tile.TileContext(nc) as tc:
        import contextlib

        with contextlib.ExitStack() as ctx:
            consts = ctx.enter_context(tc.tile_pool(name="consts", bufs=1))
            big = ctx.enter_context(tc.tile_pool(name="big", bufs=1))
            statsp = ctx.enter_context(tc.tile_pool(name="stats", bufs=1))
            gnp = ctx.enter_context(tc.tile_pool(name="gn", bufs=1))
            scr = ctx.enter_context(tc.tile_pool(name="scr", bufs=2))
            gsp = ctx.enter_context(tc.tile_pool(name="gs", bufs=5))
            gbb_p = ctx.enter_context(tc.tile_pool(name="gbb", bufs=3))
            fp = ctx.enter_context(tc.tile_pool(name="f", bufs=3))
            outp = ctx.enter_context(tc.tile_pool(name="out", bufs=4))
            convps = ctx.enter_context(
                tc.tile_pool(name="convps", bufs=CONV_GROUP, space="PSUM")
            )
            auxps = ctx.enter_context(tc.tile_pool(name="auxps", bufs=1, space="PSUM"))
            vsps = ctx.enter_context(tc.tile_pool(name="vsps", bufs=1, space="PSUM"))

            # ---- big buffers (declared first so loads can start early) -----
            xpad = big.tile([128, KC, PHW], f32r, tag="xpad")
            h1pad = big.tile([128, KC, PHW], f32r, tag="h1pad")
            h2raw = big.tile([128, MC, HW], f32, tag="h2raw")

            # weights for conv1: split per (kc, mc) for earlier availability
            def load_weights(w_dram, skip00=False):
                wt = big.tile([128, KC, 9, MC, 128], f32r, tag="wt", name="wt")
                for mc in range(MC):
                    for kc in range(KC):
                        if skip00 and mc == 0 and kc == 0:
                            continue
                        nc.sync.dma_start(
                            out=wt[:, kc, :, mc, :],
                            in_=w_dram[
                                :, kc * 128 : (kc + 1) * 128,
                                mc * 128 : (mc + 1) * 128,
                            ]
                            .rearrange("t p o -> p t o")
                            .bitcast(f32r),
                        )
                return wt

            # zero the padded-image borders with tiny on-chip copies (DMAs
            # here would be thousands of single-element descriptors)
            zeros_sb = consts.tile([128, PW], f32, tag="zeros")
            nc.gpsimd.memset(zeros_sb, 0.0)

            def zero_borders(buf):
                for kc in range(KC):
                    v = _pad_view(buf, kc)
                    nc.gpsimd.tensor_copy(out=v[:, 0, :], in_=zeros_sb[:, :PW])
                    nc.gpsimd.tensor_copy(out=v[:, PW - 1, :], in_=zeros_sb[:, :PW])
                    nc.gpsimd.tensor_copy(
                        out=v[:, 1 : PW - 1, 0:1].rearrange("p r one -> p (r one)"),
                        in_=zeros_sb[:, :H],
                    )
                    nc.gpsimd.tensor_copy(
                        out=v[:, 1 : PW - 1, PW - 1 : PW].rearrange(
                            "p r one -> p (r one)"
                        ),
                        in_=zeros_sb[:, :H],
                    )

            zero_borders(xpad)

            # weights for the first output chunk, then x straight into the
            # padded layout in row bands (kc-interleaved so early conv tiles
            # unblock first), then the second chunk's weights.
            XROWS = [(0, 6), (6, 21), (27, 27), (54, 26)]
            wt1 = big.tile([128, KC, 9, MC, 128], f32r, tag="wt", name="wt1")

            def load_w1(kc, mc):
                nc.sync.dma_start(
                    out=wt1[:, kc, :, mc, :],
                    in_=w1_h[
                        :, kc * 128 : (kc + 1) * 128,
                        mc * 128 : (mc + 1) * 128,
                    ]
                    .rearrange("t p o -> p t o")
                    .bitcast(f32r),
                )

            def load_x(kc, r0, nr):
                nc.sync.dma_start(
                    out=_pad_view(xpad, kc)[:, 1 + r0 : 1 + r0 + nr, 1 : 1 + W],
                    in_=x_h[kc * 128 : (kc + 1) * 128, r0 * W : (r0 + nr) * W]
                    .rearrange("p (r c) -> p r c", c=W)
                    .bitcast(f32r),
                )

            # critical path for the first 9 matmuls: w1(kc0,mc0) + x(kc0) rows
            load_w1(0, 0)
            load_x(0, *XROWS[0])
            load_w1(1, 0)
            load_x(1, *XROWS[0])
            load_x(0, *XROWS[1])
            load_x(1, *XROWS[1])
            for r0, nr in XROWS[2:]:
                for kc in range(KC):
                    load_x(kc, r0, nr)
            load_w1(0, 1)
            load_w1(1, 1)
            # prefetch conv2's first-needed weight slice (kc0, mc0) into its
            # own small buffer so conv2 isn't gated on the wt-slot WAR
            wt2a = big.tile([128, 9, 128], f32r, tag="wt2a", name="wt2a")
            nc.sync.dma_start(
                out=wt2a,
                in_=w2_h[:, 0:128, 0:128].rearrange("t p o -> p t o").bitcast(f32r),
            )

            # ---- constants (small DMAs on gpsimd's cheap queue) ------------
            mask_sb = consts.tile([128, 128], f32, tag="mask")
            nc.gpsimd.dma_start(out=mask_sb, in_=mask_h[:, :])
            ones_sb = consts.tile([1, 128], f32r, tag="ones")
            nc.gpsimd.dma_start(out=ones_sb, in_=ones_h[:, :].bitcast(f32r))
            ident_sb = consts.tile([128, 128], f32r, tag="ident")
            nc.gpsimd.dma_start(out=ident_sb, in_=ident_h[:, :].bitcast(f32r))
            gatew_sb = consts.tile([128, KC], f32r, tag="gatew")
            nc.gpsimd.dma_start(
                out=gatew_sb,
                in_=gatew_h[:].rearrange("(k p) -> p k", p=128).bitcast(f32r),
            )
            gn_sb = {}
            for name in ("gn1w", "gn1b", "gn2w", "gn2b"):
                t = consts.tile([128, MC], f32, tag=name, name=name)
                nc.gpsimd.dma_start(
                    out=t, in_=gn_h[name][:].rearrange("(m p) -> p m", p=128)
                )
                gn_sb[name] = t
            eps_sb = consts.tile([128, 1], f32, tag="eps")
            nc.vector.memset(eps_sb, EPS)
            gbias_sb = consts.tile([1, 1], f32, tag="gbias")
            nc.vector.memset(gbias_sb, gate_bias)

            s1_sb = gnp.tile([128, MC], f32, tag="s1")
            s2_sb = gnp.tile([128, MC], f32, tag="s2")
            t1_sb = gnp.tile([128, MC], f32, tag="t1")
            t2_sb = gnp.tile([128, MC], f32, tag="t2")
            s_sb = {1: s1_sb, 2: s2_sb}
            t_sb = {1: t1_sb, 2: t2_sb}

            # zero h1pad borders (needed only after conv1 starts evacuating)
            zero_borders(h1pad)

            # ---- one conv layer + GN stats --------------------------------
            def conv(inbuf, wt, evac_to_pad, outbuf, stats_tiles, mc_done=None,
                     wt_sel=None):
                if wt_sel is None:
                    def wt_sel(w, kc, tap, mc):
                        return w[:, kc, tap, mc, :]
                for mc in range(MC):
                    ti = 0
                    while ti < NTILES:
                        group = TILES[ti : ti + CONV_GROUP]
                        psums = [
                            convps.tile([128, R * W], f32, name="cps", tag="cps")
                            for _ in group
                        ]
                        pairs = [(kc, tap) for kc in range(KC) for tap in range(9)]
                        if len(group) > 1:
                            emit = [
                                (gi, pi)
                                for pi in range(len(pairs) - 3)
                                for gi in range(len(group))
                            ]
                            emit += [(0, len(pairs) - 3), (0, len(pairs) - 2), (0, len(pairs) - 1)]
                            emit += [
                                (gi, pi)
                                for pi in range(len(pairs) - 3, len(pairs))
                                for gi in range(1, len(group))
                            ]
                        else:
                            emit = [(0, pi) for pi in range(len(pairs))]
                        for gi, pi in emit:
                            kc, tap = pairs[pi]
                            ky, kx = divmod(tap, 3)
                            y0, rr = group[gi]
                            rhs = _pad_view(inbuf, kc)[
                                :, y0 + ky : y0 + ky + rr, kx : kx + W
                            ]
                            nc.tensor.matmul(
                                out=psums[gi][:, : rr * W],
                                lhsT=wt_sel(wt, kc, tap, mc),
                                rhs=rhs,
                                start=(pi == 0),
                                stop=(pi == len(pairs) - 1),
                            )
                        for gi, (y0, rr) in enumerate(group):
                            nt = rr * W
                            nc.vector.bn_stats(
                                out=stats_tiles[mc][:, ti + gi, :],
                                in_=psums[gi][:, :nt],
                            )
                            if evac_to_pad:
                                dst = _pad_view(outbuf, mc)[
                                    :, 1 + y0 : 1 + y0 + rr, 1 : 1 + W
                                ]
                            else:
                                dst = outbuf[:, mc, y0 * W : y0 * W + nt].rearrange(
                                    "p (r c) -> p r c", c=W
                                )
                            nc.vector.tensor_copy(
                                out=dst,
                                in_=psums[gi][:, :nt].rearrange(
                                    "p (r c) -> p r c", c=W
                                ),
                            )
                        ti += len(group)
                    if mc_done is not None:
                        mc_done(mc)

            # ---- GN stats -> per-channel scale/bias ------------------------
            def gn_scale_bias(stats_tiles, gw, gb, s_out, t_out, mc=0):
                if True:
                    mv = scr.tile([128, 2], f32, tag="mv", name="mv")
                    nc.vector.bn_aggr(out=mv, in_=stats_tiles[mc])
                    sc = scr.tile([128, 2], f32, tag="sc", name="sc")
                    nc.vector.tensor_copy(out=sc[:, 0:1], in_=mv[:, 0:1])
                    nc.vector.tensor_tensor(
                        out=sc[:, 1:2], in0=mv[:, 0:1], in1=mv[:, 0:1], op=ALU.mult
                    )
                    nc.vector.tensor_add(out=sc[:, 1:2], in0=sc[:, 1:2], in1=mv[:, 1:2])
                    gp = auxps.tile([128, 2], f32, name="gp", tag="aux")
                    nc.tensor.matmul(out=gp, lhsT=mask_sb, rhs=sc, start=True, stop=True)
                    gps = scr.tile([128, 2], f32, tag="gps", name="gps")
                    nc.vector.tensor_copy(out=gps, in_=gp)
                    # var_g = Ex2_g - mean_g^2 ; rstd = 1/sqrt(var_g+eps)
                    vg = scr.tile([128, 3], f32, tag="vg", name="vg")
                    nc.vector.tensor_tensor(
                        out=vg[:, 0:1], in0=gps[:, 0:1], in1=gps[:, 0:1], op=ALU.mult
                    )
                    nc.vector.tensor_sub(out=vg[:, 0:1], in0=gps[:, 1:2], in1=vg[:, 0:1])
                    nc.scalar.activation(
                        out=vg[:, 1:2], in_=vg[:, 0:1], func=AF.Sqrt, bias=eps_sb
                    )
                    nc.vector.reciprocal(out=vg[:, 1:2], in_=vg[:, 1:2])
                    nc.vector.tensor_mul(
                        out=s_out[:, mc : mc + 1], in0=gw[:, mc : mc + 1], in1=vg[:, 1:2]
                    )
                    nc.vector.tensor_tensor(
                        out=vg[:, 2:3],
                        in0=gps[:, 0:1],
                        in1=s_out[:, mc : mc + 1],
                        op=ALU.mult,
                    )
                    nc.vector.tensor_sub(
                        out=t_out[:, mc : mc + 1], in0=gb[:, mc : mc + 1], in1=vg[:, 2:3]
                    )

            # ================= conv1 =================
            stats1 = [
                statsp.tile([128, NTILES, 6], f32, name=f"st1_{mc}", tag=f"st{mc}")
                for mc in range(MC)
            ]
            HALF = H // 2

            def gn1_done(mc):
                # stats -> scale/bias for this chunk, then normalize+ReLU its
                # padded interior in place (ACT top half, DVE bottom half) —
                # all while the other chunk's conv matmuls keep PE busy.
                gn_scale_bias(stats1, gn_sb["gn1w"], gn_sb["gn1b"],
                              s_sb[1], t_sb[1], mc=mc)
                vt = _pad_view(h1pad, mc)[:, 1 : 1 + HALF, 1 : 1 + W]
                nc.scalar.activation(
                    out=vt, in_=vt.bitcast(f32), func=AF.Relu,
                    bias=t_sb[1][:, mc : mc + 1], scale=s_sb[1][:, mc : mc + 1],
                )
                vb = _pad_view(h1pad, mc)[:, 1 + HALF : 1 + H, 1 : 1 + W]
                nc.vector.tensor_scalar(
                    out=vb, in0=vb.bitcast(f32),
                    scalar1=s_sb[1][:, mc : mc + 1],
                    scalar2=t_sb[1][:, mc : mc + 1],
                    op0=ALU.mult, op1=ALU.add,
                )
                nc.vector.tensor_scalar_max(out=vb, in0=vb.bitcast(f32), scalar1=0.0)

            conv(xpad, wt1, True, h1pad, stats1, mc_done=gn1_done)

            # ================= conv2 =================
            wt2 = load_weights(w2_h, skip00=True)

            def wt2_sel(w, kc, tap, mc):
                if kc == 0 and mc == 0:
                    return wt2a[:, tap, :]
                return w[:, kc, tap, mc, :]
            stats2 = [
                statsp.tile([128, NTILES, 6], f32, name=f"st2_{mc}", tag=f"st{mc}")
                for mc in range(MC)
            ]
            strow = {}

            def gn2_done(mc):
                gn_scale_bias(stats2, gn_sb["gn2w"], gn_sb["gn2b"],
                              s_sb[2], t_sb[2], mc=mc)
                if mc == 0:
                    # PE-minimal tail for the chunk that overlaps conv2: gate
                    # row broadcast via partition-broadcast DMA, elementwise
                    # math on DVE/POOL which idle under conv2's matmuls.
                    for ti, (y0, rr) in enumerate(TILES):
                        nt = rr * W
                        gpt = auxps.tile([1, R * W], f32, name="gpt", tag="aux")
                        for kc in range(KC):
                            rhs = _pad_view(xpad, kc)[
                                :, 1 + y0 : 1 + y0 + rr, 1 : 1 + W
                            ]
                            nc.tensor.matmul(
                                out=gpt[:, :nt],
                                lhsT=gatew_sb[:, kc : kc + 1],
                                rhs=rhs,
                                start=(kc == 0),
                                stop=(kc == KC - 1),
                            )
                        gsb = gsp.tile([1, R * W], f32r, tag="gsb", name="gsb")
                        nc.scalar.activation(
                            out=gsb[:, :nt], in_=gpt[:, :nt], func=AF.Tanh,
                            bias=gbias_sb,
                        )
                        nc.vector.tensor_scalar_max(
                            out=gsb[:, :nt], in0=gsb[:, :nt], scalar1=0.0
                        )
                        gbc = vsps.tile([128, R * W], f32, name="vss", tag="vss")
                        nc.tensor.matmul(
                            out=gbc[:, :nt], lhsT=ones_sb, rhs=gsb[:, :nt],
                            start=True, stop=True,
                        )
                        h2s = h2raw[:, mc, y0 * W : y0 * W + nt]
                        xint = (
                            _pad_view(xpad, mc)[:, 1 + y0 : 1 + y0 + rr, 1 : 1 + W]
                            .bitcast(f32)
                        )
                        at = fp.tile([128, R * W], f32, tag="u", name="at")
                        nc.vector.scalar_tensor_tensor(
                            out=at[:, :nt],
                            in0=h2s,
                            scalar=s_sb[2][:, mc : mc + 1],
                            in1=gbc[:, :nt],
                            op0=ALU.mult,
                            op1=ALU.mult,
                        )
                        ft = gbb_p.tile([128, R * W], f32, tag="ft", name="ft")
                        nc.vector.scalar_tensor_tensor(
                            out=ft[:, :nt].rearrange("p (r c) -> p r c", c=W),
                            in0=gbc[:, :nt].rearrange("p (r c) -> p r c", c=W),
                            scalar=t_sb[2][:, mc : mc + 1],
                            in1=xint,
                            op0=ALU.mult,
                            op1=ALU.add,
                        )
                        ot = outp.tile([128, R * W], f32, name="ot", tag="ot")
                        nc.gpsimd.tensor_add(
                            out=ot[:, :nt], in0=at[:, :nt], in1=ft[:, :nt]
                        )
                        nc.gpsimd.tensor_relu(out=ot[:, :nt], in_=ot[:, :nt])
                        nc.sync.dma_start(
                            out=y_h[
                                mc * 128 : (mc + 1) * 128, y0 * W : y0 * W + nt
                            ],
                            in_=ot[:, :nt],
                        )
                    return

                # transpose s2/t2 columns into [1,128] rows for outer products
                for nm, col in (("s", s_sb[2]), ("t", t_sb[2])):
                    tp = auxps.tile([1, 128], f32, name=f"tp{nm}{mc}", tag="aux")
                    nc.tensor.transpose(
                        tp, col[:, mc : mc + 1], ident_sb.bitcast(f32)
                    )
                    row = gnp.tile([1, 128], f32r, tag=f"{nm}row{mc}", name=f"{nm}row{mc}")
                    nc.vector.tensor_copy(out=row, in_=tp)
                    strow[(nm, mc)] = row
                # out = relu(s2*h2*g + t2*g + x), in batches of 4 tiles:
                # all gates (ACT stays on the Tanh table), then all combines.
                # vst accumulates t2xg, +x, then +u (identity matmuls), so
                # DVE does one pass (u = h2*vss) and ACT one Relu evac.
                BT = 4
                for t0i in range(0, NTILES, BT):
                    batch = list(enumerate(TILES))[t0i : t0i + BT]
                    gsbs = []
                    for ti, (y0, rr) in batch:
                        nt = rr * W
                        gpt = auxps.tile([1, R * W], f32, name="gpt", tag="aux")
                        for kc in range(KC):
                            rhs = _pad_view(xpad, kc)[
                                :, 1 + y0 : 1 + y0 + rr, 1 : 1 + W
                            ]
                            nc.tensor.matmul(
                                out=gpt[:, :nt],
                                lhsT=gatew_sb[:, kc : kc + 1],
                                rhs=rhs,
                                start=(kc == 0),
                                stop=(kc == KC - 1),
                            )
                        gsb = gsp.tile([1, R * W], f32r, tag="gsb", name="gsb")
                        nc.scalar.activation(
                            out=gsb[:, :nt], in_=gpt[:, :nt], func=AF.Tanh,
                            bias=gbias_sb,
                        )
                        nc.vector.tensor_scalar_max(
                            out=gsb[:, :nt], in0=gsb[:, :nt], scalar1=0.0
                        )
                        gsbs.append(gsb)
                    for bi, (ti, (y0, rr)) in enumerate(batch):
                        nt = rr * W
                        gsb = gsbs[bi]
                        h2s = h2raw[:, mc, y0 * W : y0 * W + nt]
                        xint = _pad_view(xpad, mc)[
                            :, 1 + y0 : 1 + y0 + rr, 1 : 1 + W
                        ]
                        vss = convps.tile([128, R * W], f32, name="vss", tag="cps")
                        nc.tensor.matmul(
                            out=vss[:, :nt], lhsT=strow[("s", mc)], rhs=gsb[:, :nt],
                            start=True, stop=True,
                        )
                        ut = fp.tile([128, R * W], f32r, tag="u", name="ut")
                        nc.vector.tensor_tensor(
                            out=ut[:, :nt], in0=h2s, in1=vss[:, :nt], op=ALU.mult
                        )
                        vst = convps.tile([128, R * W], f32, name="vst", tag="cps")
                        nc.tensor.matmul(
                            out=vst[:, :nt], lhsT=strow[("t", mc)], rhs=gsb[:, :nt],
                            start=True, stop=False,
                        )
                        nc.tensor.matmul(
                            out=vst[:, :nt].rearrange("p (r c) -> p r c", c=W),
                            lhsT=ident_sb,
                            rhs=xint,
                            start=False, stop=False,
                        )
                        nc.tensor.matmul(
                            out=vst[:, :nt],
                            lhsT=ident_sb,
                            rhs=ut[:, :nt],
                            start=False, stop=True,
                        )
                        ot = outp.tile([128, R * W], f32, name="ot", tag="ot")
                        nc.scalar.activation(
                            out=ot[:, :nt], in_=vst[:, :nt], func=AF.Relu
                        )
                        nc.sync.dma_start(
                            out=y_h[mc * 128 : (mc + 1) * 128, y0 * W : y0 * W + nt],
                            in_=ot[:, :nt],
                        )

            conv(h1pad, wt2, False, h2raw, stats2, mc_done=gn2_done,
                 wt_sel=wt2_sel)

            # (final combine is emitted per-chunk from gn2_done so chunk 0's
            # tail overlaps chunk 1's conv2 matmuls)

    _split_multi_waits(nc)
    return nc


def _host_prep(w1, w2, gate_w):
    w1t = np.ascontiguousarray(np.transpose(w1, (2, 3, 1, 0)).reshape(9, C, C))
    w2t = np.ascontiguousarray(np.transpose(w2, (2, 3, 1, 0)).reshape(9, C, C))
    gw = np.ascontiguousarray(gate_w.reshape(C))
    mask = np.zeros((128, 128), np.float32)
    for g in range(128 // GROUP):
        mask[g * GROUP : (g + 1) * GROUP, g * GROUP : (g + 1) * GROUP] = 1.0 / GROUP
    ones = np.ones((1, 128), np.float32)
    return w1t, w2t, gw, mask, ones


def make_in_maps(x, w1, gn1_w, gn1_b, w2, gn2_w, gn2_b, gate_w, gate_b):
    x = np.asarray(x, np.float32)
    w1t, w2t, gw, mask, ones = _host_prep(
        np.asarray(w1, np.float32), np.asarray(w2, np.float32),
        np.asarray(gate_w, np.float32),
    )
    shared = {
        "w1t": w1t,
        "w2t": w2t,
        "gn1w": np.asarray(gn1_w, np.float32),
        "gn1b": np.asarray(gn1_b, np.float32),
        "gn2w": np.asarray(gn2_w, np.float32),
        "gn2b": np.asarray(gn2_b, np.float32),
        "gatew": gw,
        "mask": mask,
        "ones": ones,
        "ident": np.eye(128, dtype=np.float32),
    }
    return [
        {"x": np.ascontiguousarray(x[b].reshape(C, HW)), **shared} for b in range(B)
    ]


def kernel(x, w1, gn1_w, gn1_b, w2, gn2_w, gn2_b, gate_w, gate_b):
    gate_bias = float(np.asarray(gate_b).reshape(-1)[0])
    nc = build_program(gate_bias)
    in_maps = make_in_maps(
        x, w1, gn1_w, gn1_b, w2, gn2_w, gn2_b, gate_w, gate_b
    )
    res = run_bass_kernel_spmd(nc, in_maps, core_ids=list(range(B)))
    out = np.stack(
        [res.results[b]["y"].reshape(C, H, W) for b in range(B)], axis=0
    )
    return out


# revision 55
# speedup vs baseline: 1.9146x; 1.9146x over previous
"""DynamicBottleneck Trainium2 kernel.

Data-parallel over batch: each of 8 NeuronCores computes one sample of
x: [8, 256, 80, 80] through conv3x3 -> GN -> ReLU -> conv3x3 -> GN,
a 1x1-conv spatial gate (ReTanH) on the input, gating + residual + ReLU.

Per-core layout: channels on partitions (256 = 2 chunks of 128), spatial
pixels on the free dim. Inputs/hidden live in SBUF as zero-padded 82x82
images so each of the 9 conv taps is a strided AP read; the conv is 9
accumulated f32r matmuls per (out-chunk, in-chunk) into PSUM. GroupNorm
stats come from bn_stats/bn_aggr per channel + a block-mask matmul for the
cross-partition (8-channel group) reduction.
"""

import sys

sys.path.insert(0, "/opt/trn_rl_repo")

import numpy as np
import concourse.bass as bass
import concourse.tile as tile
from concourse import mybir
from concourse.bass_utils import run_bass_kernel_spmd

f32 = mybir.dt.float32
f32r = mybir.dt.float32r
AF = mybir.ActivationFunctionType
ALU = mybir.AluOpType

B, C, H, W = 8, 256, 80, 80
HW = H * W          # 6400
PW = W + 2          # 82
PHW = PW * PW       # 6724
MC = C // 128       # output-channel chunks
KC = C // 128       # input-channel chunks
GROUP = 8           # channels per GN group (256 / 32)
R = 5               # image rows per spatial tile -> N=400 per matmul
TILES = [(y0, R) for y0 in range(0, H, R)]   # 16 equal tiles
NTILES = len(TILES)
CONV_GROUP = 5      # PSUM banks cycled by conv accumulation
XBANDS = 2          # row-bands for the staged x load
EPS = 1e-5

# ---------------------------------------------------------------------------
# walrus in this container accepts only ONE sem-wait per instruction; tile
# attaches several. Rewrite blocks so extra waits ride on single-wait NOPs.
_ENGINE_ATTR = {
    "EngineType.PE": "tensor",
    "EngineType.Activation": "scalar",
    "EngineType.DVE": "vector",
    "EngineType.Pool": "gpsimd",
    "EngineType.SP": "sync",
}


def _fresh_nop(nc, engine):
    bi = getattr(nc, _ENGINE_ATTR[str(engine)]).nop(nofuse=True)
    cur = nc.cur_bb.bb
    insts = cur.instructions
    assert insts and insts[-1].name == bi.ins.name
    cur.instructions = insts[:-1]
    return bi.ins


def _split_multi_waits(nc):
    for f in nc.m.functions:
        for bb in f.blocks:
            insts = bb.instructions
            if not any(
                i.sync_info is not None and len(i.sync_info.on_wait) > 1
                for i in insts
            ):
                continue
            out = []
            for inst in insts:
                si = inst.sync_info
                if si is not None and len(si.on_wait) > 1:
                    waits = list(si.on_wait)
                    for w in waits[:-1]:
                        nop = _fresh_nop(nc, inst.engine)
                        nop.sync_info = mybir.SyncInfo(on_wait=[w], on_update=[])
                        out.append(nop)
                    inst.sync_info = mybir.SyncInfo(
                        on_wait=[waits[-1]], on_update=list(si.on_update)
                    )
                out.append(inst)
            bb.instructions = out


# ---------------------------------------------------------------------------


def _pad_view(buf_ap, kc):
    """[128, PHW] padded-image chunk as [128, 82, 82]."""
    return buf_ap[:, kc, :].rearrange("p (r c) -> p r c", c=PW)


def build_program(gate_bias: float):
    nc = bass.Bass()

    x_h = nc.declare_dram_parameter("x", [C, HW], f32, isOutput=False)
    w1_h = nc.declare_dram_parameter("w1t", [9, C, C], f32, isOutput=False)
    w2_h = nc.declare_dram_parameter("w2t", [9, C, C], f32, isOutput=False)
    gn_h = {}
    for name in ("gn1w", "gn1b", "gn2w", "gn2b"):
        gn_h[name] = nc.declare_dram_parameter(name, [C], f32, isOutput=False)
    gatew_h = nc.declare_dram_parameter("gatew", [C], f32, isOutput=False)
    mask_h = nc.declare_dram_parameter("mask", [128, 128], f32, isOutput=False)
    ones_h = nc.declare_dram_parameter("ones", [1, 128], f32, isOutput=False)
    ident_h = nc.declare_dram_parameter("ident", [128, 128], f32, isOutput=False)
    y_h = nc.declare_dram_parameter("y", [C, HW], f32, isOutput=True)

    with tile.TileContext(nc) as tc:
        import contextlib

        with contextlib.ExitStack() as ctx:
            consts = ctx.enter_context(tc.tile_pool(name="consts", bufs=1))
            big = ctx.enter_context(tc.tile_pool(name="big", bufs=1))
            statsp = ctx.enter_context(tc.tile_pool(name="stats", bufs=1))
            gnp = ctx.enter_context(tc.tile_pool(name="gn", bufs=1))
            scr = ctx.enter_context(tc.tile_pool(name="scr", bufs=2))
            gsp = ctx.enter_context(tc.tile_pool(name="gs", bufs=5))
            gbb_p = ctx.enter_context(tc.tile_pool(name="gbb", bufs=3))
            fp = ctx.enter_context(tc.tile_pool(name="f", bufs=3))
            outp = ctx.enter_context(tc.tile_pool(name="out", bufs=4))
            convps = ctx.enter_context(
                tc.tile_pool(name="convps", bufs=CONV_GROUP, space="PSUM")
            )
            auxps = ctx.enter_context(tc.tile_pool(name="auxps", bufs=1, space="PSUM"))
            vsps = ctx.enter_context(tc.tile_pool(name="vsps", bufs=1, space="PSUM"))

            # ---- big buffers (declared first so loads can start early) -----
            xpad = big.tile([128, KC, PHW], f32r, tag="xpad")
            h1pad = big.tile([128, KC, PHW], f32r, tag="h1pad")
            h2raw = big.tile([128, MC, HW], f32, tag="h2raw")

            # weights for conv1: split per (kc, mc) for earlier availability
            def load_weights(w_dram, skip00=False):
                wt = big.tile([128, KC, 9, MC, 128], f32r, tag="wt", name="wt")
                for mc in range(MC):
                    for kc in range(KC):
                        if skip00 and mc == 0 and kc == 0:
                            continue
                        nc.sync.dma_start(
                            out=wt[:, kc, :, mc, :],
                            in_=w_dram[
                                :, kc * 128 : (kc + 1) * 128,
                                mc * 128 : (mc + 1) * 128,
                            ]
                            .rearrange("t p o -> p t o")
                            .bitcast(f32r),
                        )
                return wt

            # zero the padded-image borders with tiny on-chip copies (DMAs
            # here would be thousands of single-element descriptors)
            zeros_sb = consts.tile([128, PW], f32, tag="zeros")
            nc.gpsimd.memset(zeros_sb, 0.0)

            def zero_borders(buf):
                for kc in range(KC):
                    v = _pad_view(buf, kc)
                    nc.gpsimd.tensor_copy(out=v[:, 0, :], in_=zeros_sb[:, :PW])
                    nc.gpsimd.tensor_copy(out=v[:, PW - 1, :], in_=zeros_sb[:, :PW])
                    nc.gpsimd.tensor_copy(
                        out=v[:, 1 : PW - 1, 0:1].rearrange("p r one -> p (r one)"),
                        in_=zeros_sb[:, :H],
                    )
                    nc.gpsimd.tensor_copy(
                        out=v[:, 1 : PW - 1, PW - 1 : PW].rearrange(
                            "p r one -> p (r one)"
                        ),
                        in_=zeros_sb[:, :H],
                    )

            zero_borders(xpad)

            # weights for the first output chunk, then x straight into the
            # padded layout in row bands (kc-interleaved so early conv tiles
            # unblock first), then the second chunk's weights.
            XROWS = [(0, 6), (6, 21), (27, 27), (54, 26)]
            wt1 = big.tile([128, KC, 9, MC, 128], f32r, tag="wt", name="wt1")

            def load_w1(kc, mc):
                nc.sync.dma_start(
                    out=wt1[:, kc, :, mc, :],
                    in_=w1_h[
                        :, kc * 128 : (kc + 1) * 128,
                        mc * 128 : (mc + 1) * 128,
                    ]
                    .rearrange("t p o -> p t o")
                    .bitcast(f32r),
                )

            def load_x(kc, r0, nr):
                nc.sync.dma_start(
                    out=_pad_view(xpad, kc)[:, 1 + r0 : 1 + r0 + nr, 1 : 1 + W],
                    in_=x_h[kc * 128 : (kc + 1) * 128, r0 * W : (r0 + nr) * W]
                    .rearrange("p (r c) -> p r c", c=W)
                    .bitcast(f32r),
                )

            # critical path for the first 9 matmuls: w1(kc0,mc0) + x(kc0) rows
            load_w1(0, 0)
            load_x(0, *XROWS[0])
            load_w1(1, 0)
            load_x(1, *XROWS[0])
            load_x(0, *XROWS[1])
            load_x(1, *XROWS[1])
            for r0, nr in XROWS[2:]:
                for kc in range(KC):
                    load_x(kc, r0, nr)
            load_w1(0, 1)
            load_w1(1, 1)
            # prefetch conv2's first-needed weight slice (kc0, mc0) into its
            # own small buffer so conv2 isn't gated on the wt-slot WAR
            wt2a = big.tile([128, 9, 128], f32r, tag="wt2a", name="wt2a")
            nc.sync.dma_start(
                out=wt2a,
                in_=w2_h[:, 0:128, 0:128].rearrange("t p o -> p t o").bitcast(f32r),
            )

            # ---- constants (small DMAs on gpsimd's cheap queue) ------------
            mask_sb = consts.tile([128, 128], f32, tag="mask")
            nc.gpsimd.dma_start(out=mask_sb, in_=mask_h[:, :])
            ones_sb = consts.tile([1, 128], f32r, tag="ones")
            nc.gpsimd.dma_start(out=ones_sb, in_=ones_h[:, :].bitcast(f32r))
            ident_sb = consts.tile([128, 128], f32r, tag="ident")
            nc.gpsimd.dma_start(out=ident_sb, in_=ident_h[:, :].bitcast(f32r))
            gatew_sb = consts.tile([128, KC], f32r, tag="gatew")
            nc.gpsimd.dma_start(
                out=gatew_sb,
                in_=gatew_h[:].rearrange("(k p) -> p k", p=128).bitcast(f32r),
            )
            gn_sb = {}
            for name in ("gn1w", "gn1b", "gn2w", "gn2b"):
                t = consts.tile([128, MC], f32, tag=name, name=name)
                nc.gpsimd.dma_start(
                    out=t, in_=gn_h[name][:].rearrange("(m p) -> p m", p=128)
                )
                gn_sb[name] = t
            eps_sb = consts.tile([128, 1], f32, tag="eps")
            nc.vector.memset(eps_sb, EPS)
            gbias_sb = consts.tile([1, 1], f32, tag="gbias")
            nc.vector.memset(gbias_sb, gate_bias)

            s1_sb = gnp.tile([128, MC], f32, tag="s1")
            s2_sb = gnp.tile([128, MC], f32, tag="s2")
            t1_sb = gnp.tile([128, MC], f32, tag="t1")
            t2_sb = gnp.tile([128, MC], f32, tag="t2")
            s_sb = {1: s1_sb, 2: s2_sb}
            t_sb = {1: t1_sb, 2: t2_sb}

            # zero h1pad borders (needed only after conv1 starts evacuating)
            zero_borders(h1pad)

            # ---- one conv layer + GN stats --------------------------------
            def conv(inbuf, wt, evac_to_pad, outbuf, stats_tiles, mc_done=None,
                     wt_sel=None):
                if wt_sel is None:
                    def wt_sel(w, kc, tap, mc):
                        return w[:, kc, tap, mc, :]
                for mc in range(MC):
                    ti = 0
                    while ti < NTILES:
                        group = TILES[ti : ti + CONV_GROUP]
                        psums = [
                            convps.tile([128, R * W], f32, name="cps", tag="cps")
                            for _ in group
                        ]
                        pairs = [(kc, tap) for kc in range(KC) for tap in range(9)]
                        if len(group) > 1:
                            emit = [
                                (gi, pi)
                                for pi in range(len(pairs) - 3)
                                for gi in range(len(group))
                            ]
                            emit += [(0, len(pairs) - 3), (0, len(pairs) - 2), (0, len(pairs) - 1)]
                            emit += [
                                (gi, pi)
                                for pi in range(len(pairs) - 3, len(pairs))
                                for gi in range(1, len(group))
                            ]
                        else:
                            emit = [(0, pi) for pi in range(len(pairs))]
                        for gi, pi in emit:
                            kc, tap = pairs[pi]
                            ky, kx = divmod(tap, 3)
                            y0, rr = group[gi]
                            rhs = _pad_view(inbuf, kc)[
                                :, y0 + ky : y0 + ky + rr, kx : kx + W
                            ]
                            nc.tensor.matmul(
                                out=psums[gi][:, : rr * W],
                                lhsT=wt_sel(wt, kc, tap, mc),
                                rhs=rhs,
                                start=(pi == 0),
                                stop=(pi == len(pairs) - 1),
                            )
                        for gi, (y0, rr) in enumerate(group):
                            nt = rr * W
                            nc.vector.bn_stats(
                                out=stats_tiles[mc][:, ti + gi, :],
                                in_=psums[gi][:, :nt],
                            )
                            if evac_to_pad:
                                dst = _pad_view(outbuf, mc)[
                                    :, 1 + y0 : 1 + y0 + rr, 1 : 1 + W
                                ]
                            else:
                                dst = outbuf[:, mc, y0 * W : y0 * W + nt].rearrange(
                                    "p (r c) -> p r c", c=W
                                )
                            nc.vector.tensor_copy(
                                out=dst,
                                in_=psums[gi][:, :nt].rearrange(
                                    "p (r c) -> p r c", c=W
                                ),
                            )
                        ti += len(group)
                    if mc_done is not None:
                        mc_done(mc)

            # ---- GN stats -> per-channel scale/bias ------------------------
            def gn_scale_bias(stats_tiles, gw, gb, s_out, t_out, mc=0):
                if True:
                    mv = scr.tile([128, 2], f32, tag="mv", name="mv")
                    nc.vector.bn_aggr(out=mv, in_=stats_tiles[mc])
                    sc = scr.tile([128, 2], f32, tag="sc", name="sc")
                    nc.vector.tensor_copy(out=sc[:, 0:1], in_=mv[:, 0:1])
                    nc.vector.tensor_tensor(
                        out=sc[:, 1:2], in0=mv[:, 0:1], in1=mv[:, 0:1], op=ALU.mult
                    )
                    nc.vector.tensor_add(out=sc[:, 1:2], in0=sc[:, 1:2], in1=mv[:, 1:2])
                    gp = auxps.tile([128, 2], f32, name="gp", tag="aux")
                    nc.tensor.matmul(out=gp, lhsT=mask_sb, rhs=sc, start=True, stop=True)
                    gps = scr.tile([128, 2], f32, tag="gps", name="gps")
                    nc.vector.tensor_copy(out=gps, in_=gp)
                    # var_g = Ex2_g - mean_g^2 ; rstd = 1/sqrt(var_g+eps)
                    vg = scr.tile([128, 3], f32, tag="vg", name="vg")
                    nc.vector.tensor_tensor(
                        out=vg[:, 0:1], in0=gps[:, 0:1], in1=gps[:, 0:1], op=ALU.mult
                    )
                    nc.vector.tensor_sub(out=vg[:, 0:1], in0=gps[:, 1:2], in1=vg[:, 0:1])
                    nc.scalar.activation(
                        out=vg[:, 1:2], in_=vg[:, 0:1], func=AF.Sqrt, bias=eps_sb
                    )
                    nc.vector.reciprocal(out=vg[:, 1:2], in_=vg[:, 1:2])
                    nc.vector.tensor_mul(
                        out=s_out[:, mc : mc + 1], in0=gw[:, mc : mc + 1], in1=vg[:, 1:2]
                    )
                    nc.vector.tensor_tensor(
                        out=vg[:, 2:3],
                        in0=gps[:, 0:1],
                        in1=s_out[:, mc : mc + 1],
                        op=ALU.mult,
                    )
                    nc.vector.tensor_sub(
                        out=t_out[:, mc : mc + 1], in0=gb[:, mc : mc + 1], in1=vg[:, 2:3]
                    )

            # ================= conv1 =================
            stats1 = [
                statsp.tile([128, NTILES, 6], f32, name=f"st1_{mc}", tag=f"st{mc}")
                for mc in range(MC)
            ]
            HALF = H // 2

            def gn1_done(mc):
                # stats -> scale/bias for this chunk, then normalize+ReLU its
                # padded interior in place (ACT top half, DVE bottom half) —
                # all while the other chunk's conv matmuls keep PE busy.
                gn_scale_bias(stats1, gn_sb["gn1w"], gn_sb["gn1b"],
                              s_sb[1], t_sb[1], mc=mc)
                vt = _pad_view(h1pad, mc)[:, 1 : 1 + HALF, 1 : 1 + W]
                nc.scalar.activation(
                    out=vt, in_=vt.bitcast(f32), func=AF.Relu,
                    bias=t_sb[1][:, mc : mc + 1], scale=s_sb[1][:, mc : mc + 1],
                )
                vb = _pad_view(h1pad, mc)[:, 1 + HALF : 1 + H, 1 : 1 + W]
                nc.vector.tensor_scalar(
                    out=vb, in0=vb.bitcast(f32),
                    scalar1=s_sb[1][:, mc : mc + 1],
                    scalar2=t_sb[1][:, mc : mc + 1],
                    op0=ALU.mult, op1=ALU.add,
                )
                nc.vector.tensor_scalar_max(out=vb, in0=vb.bitcast(f32), scalar1=0.0)

            conv(xpad, wt1, True, h1pad, stats1, mc_done=gn1_done)

            # ================= conv2 =================
            wt2 = load_weights(w2_h, skip00=True)

            def wt2_sel(w, kc, tap, mc):
                if kc == 0 and mc == 0:
                    return wt2a[:, tap, :]
                return w[:, kc, tap, mc, :]
            stats2 = [
                statsp.tile([128, NTILES, 6], f32, name=f"st2_{mc}", tag=f"st{mc}")
                for mc in range(MC)
            ]
            strow = {}

            def gn2_done(mc):
                gn_scale_bias(stats2, gn_sb["gn2w"], gn_sb["gn2b"],
                              s_sb[2], t_sb[2], mc=mc)
                if mc == 0:
                    # PE-minimal tail for the chunk that overlaps conv2: gate
                    # row broadcast via partition-broadcast DMA, elementwise
                    # math on DVE/POOL which idle under conv2's matmuls.
                    for ti, (y0, rr) in enumerate(TILES):
                        nt = rr * W
                        gpt = auxps.tile([1, R * W], f32, name="gpt", tag="aux")
                        for kc in range(KC):
                            rhs = _pad_view(xpad, kc)[
                                :, 1 + y0 : 1 + y0 + rr, 1 : 1 + W
                            ]
                            nc.tensor.matmul(
                                out=gpt[:, :nt],
                                lhsT=gatew_sb[:, kc : kc + 1],
                                rhs=rhs,
                                start=(kc == 0),
                                stop=(kc == KC - 1),
                            )
                        gsb = gsp.tile([1, R * W], f32r, tag="gsb", name="gsb")
                        nc.scalar.activation(
                            out=gsb[:, :nt], in_=gpt[:, :nt], func=AF.Tanh,
                            bias=gbias_sb,
                        )
                        nc.vector.tensor_scalar_max(
                            out=gsb[:, :nt], in0=gsb[:, :nt], scalar1=0.0
                        )
                        gbc = vsps.tile([128, R * W], f32, name="vss", tag="vss")
                        nc.tensor.matmul(
                            out=gbc[:, :nt], lhsT=ones_sb, rhs=gsb[:, :nt],
                            start=True, stop=True,
                        )
                        h2s = h2raw[:, mc, y0 * W : y0 * W + nt]
                        xint = (
                            _pad_view(xpad, mc)[:, 1 + y0 : 1 + y0 + rr, 1 : 1 + W]
                            .bitcast(f32)
                        )
                        at = fp.tile([128, R * W], f32, tag="u", name="at")
                        nc.vector.scalar_tensor_tensor(
                            out=at[:, :nt],
                            in0=h2s,
                            scalar=s_sb[2][:, mc : mc + 1],
                            in1=gbc[:, :nt],
                            op0=ALU.mult,
                            op1=ALU.mult,
                        )
                        ft = gbb_p.tile([128, R * W], f32, tag="ft", name="ft")
                        nc.vector.scalar_tensor_tensor(
                            out=ft[:, :nt].rearrange("p (r c) -> p r c", c=W),
                            in0=gbc[:, :nt].rearrange("p (r c) -> p r c", c=W),
                            scalar=t_sb[2][:, mc : mc + 1],
                            in1=xint,
                            op0=ALU.mult,
                            op1=ALU.add,
                        )
                        ot = outp.tile([128, R * W], f32, name="ot", tag="ot")
                        nc.gpsimd.tensor_add(
                            out=ot[:, :nt], in0=at[:, :nt], in1=ft[:, :nt]
                        )
                        nc.gpsimd.tensor_relu(out=ot[:, :nt], in_=ot[:, :nt])
                        nc.sync.dma_start(
                            out=y_h[
                                mc * 128 : (mc + 1) * 128, y0 * W : y0 * W + nt
                            ],
                            in_=ot[:, :nt],
                        )
                    return

                # transpose s2/t2 columns into [1,128] rows for outer products
                for nm, col in (("s", s_sb[2]), ("t", t_sb[2])):
                    tp = auxps.tile([1, 128], f32, name=f"tp{nm}{mc}", tag="aux")
                    nc.tensor.transpose(
                        tp, col[:, mc : mc + 1], ident_sb.bitcast(f32)
                    )
                    row = gnp.tile([1, 128], f32r, tag=f"{nm}row{mc}", name=f"{nm}row{mc}")
                    nc.vector.tensor_copy(out=row, in_=tp)
                    strow[(nm, mc)] = row
                # out = relu(s2*h2*g + t2*g + x), in batches of 4 tiles:
                # all gates (ACT stays on the Tanh table), then all combines.
                # vst accumulates t2xg, +x, then +u (identity matmuls), so
                # DVE does one pass (u = h2*vss) and ACT one Relu evac.
                BT = 4
                for t0i in range(0, NTILES, BT):
                    batch = list(enumerate(TILES))[t0i : t0i + BT]
                    gsbs = []
                    for ti, (y0, rr) in batch:
                        nt = rr * W
                        gpt = auxps.tile([1, R * W], f32, name="gpt", tag="aux")
                        for kc in range(KC):
                            rhs = _pad_view(xpad, kc)[
                                :, 1 + y0 : 1 + y0 + rr, 1 : 1 + W
                            ]
                            nc.tensor.matmul(
                                out=gpt[:, :nt],
                                lhsT=gatew_sb[:, kc : kc + 1],
                                rhs=rhs,
                                start=(kc == 0),
                                stop=(kc == KC - 1),
                            )
                        gsb = gsp.tile([1, R * W], f32r, tag="gsb", name="gsb")
                        nc.scalar.activation(
                            out=gsb[:, :nt], in_=gpt[:, :nt], func=AF.Tanh,
                            bias=gbias_sb,
                        )
                        nc.vector.tensor_scalar_max(
                            out=gsb[:, :nt], in0=gsb[:, :nt], scalar1=0.0
                        )
                        gsbs.append(gsb)
                    for bi, (ti, (y0, rr)) in enumerate(batch):
                        nt = rr * W
                        gsb = gsbs[bi]
                        h2s = h2raw[:, mc, y0 * W : y0 * W + nt]
                        xint = _pad_view(xpad, mc)[
                            :, 1 + y0 : 1 + y0 + rr, 1 : 1 + W
                        ]
                        vss = convps.tile([128, R * W], f32, name="vss", tag="cps")
                        nc.tensor.matmul(
                            out=vss[:, :nt], lhsT=strow[("s", mc)], rhs=gsb[:, :nt],
                            start=True, stop=True,
                        )
                        ut = fp.tile([128, R * W], f32r, tag="u", name="ut")
                        nc.vector.tensor_tensor(
                            out=ut[:, :nt], in0=h2s, in1=vss[:, :nt], op=ALU.mult
                        )
                        vst = convps.tile([128, R * W], f32, name="vst", tag="cps")
                        nc.tensor.matmul(
                            out=vst[:, :nt], lhsT=strow[("t", mc)], rhs=gsb[:, :nt],
                            start=True, stop=False,
                        )
                        nc.tensor.matmul(
                            out=vst[:, :nt].rearrange("p (r c) -> p r c", c=W),
                            lhsT=ident_sb,
                            rhs=xint,
                            start=False, stop=False,
                        )
                        nc.tensor.matmul(
                            out=vst[:, :nt],
                            lhsT=ident_sb,
                            rhs=ut[:, :nt],
                            start=False, stop=True,
                        )
                        ot = outp.tile([128, R * W], f32, name="ot", tag="ot")
                        nc.scalar.activation(
                            out=ot[:, :nt], in_=vst[:, :nt], func=AF.Relu
                        )
                        nc.sync.dma_start(
                            out=y_h[mc * 128 : (mc + 1) * 128, y0 * W : y0 * W + nt],
                            in_=ot[:, :nt],
                        )

            conv(h1pad, wt2, False, h2raw, stats2, mc_done=gn2_done,
                 wt_sel=wt2_sel)

            # (final combine is emitted per-chunk from gn2_done so chunk 0's
            # tail overlaps chunk 1's conv2 matmuls)

    _split_multi_waits(nc)
    return nc


def _host_prep(w1, w2, gate_w):
    w1t = np.ascontiguousarray(np.transpose(w1, (2, 3, 1, 0)).reshape(9, C, C))
    w2t = np.ascontiguousarray(np.transpose(w2, (2, 3, 1, 0)).reshape(9, C, C))
    gw = np.ascontiguousarray(gate_w.reshape(C))
    mask = np.zeros((128, 128), np.float32)
    for g in range(128 // GROUP):
        mask[g * GROUP : (g + 1) * GROUP, g * GROUP : (g + 1) * GROUP] = 1.0 / GROUP
    ones = np.ones((1, 128), np.float32)
    return w1t, w2t, gw, mask, ones


def make_in_maps(x, w1, gn1_w, gn1_b, w2, gn2_w, gn2_b, gate_w, gate_b):
    x = np.asarray(x, np.float32)
    w1t, w2t, gw, mask, ones = _host_prep(
        np.asarray(w1, np.float32), np.asarray(w2, np.float32),
        np.asarray(gate_w, np.float32),
    )
    shared = {
        "w1t": w1t,
        "w2t": w2t,
        "gn1w": np.asarray(gn1_w, np.float32),
        "gn1b": np.asarray(gn1_b, np.float32),
        "gn2w": np.asarray(gn2_w, np.float32),
        "gn2b": np.asarray(gn2_b, np.float32),
        "gatew": gw,
        "mask": mask,
        "ones": ones,
        "ident": np.eye(128, dtype=np.float32),
    }
    return [
        {"x": np.ascontiguousarray(x[b].reshape(C, HW)), **shared} for b in range(B)
    ]


def kernel(x, w1, gn1_w, gn1_b, w2, gn2_w, gn2_b, gate_w, gate_b):
    gate_bias = float(np.asarray(gate_b).reshape(-1)[0])
    nc = build_program(gate_bias)
    in_maps = make_in_maps(
        x, w1, gn1_w, gn1_b, w2, gn2_w, gn2_b, gate_w, gate_b
    )
    res = run_bass_kernel_spmd(nc, in_maps, core_ids=list(range(B)))
    out = np.stack(
        [res.results[b]["y"].reshape(C, H, W) for b in range(B)], axis=0
    )
    return out
